# revision 63
# baseline (speedup 1.0000x reference)
"""TAGConv×2 GNN discriminator on 8 Trainium2 NeuronCores.

Strategy: shard destination nodes across 8 cores (6272 rows each),
replicate the small per-hop weights, exchange propagated node features
between hops via AllGather of bf16 tables, all-reduce the pooled scalar.

v3 design vs v2 (the dominant cost was per-call host<->device traffic
through the PJRT tunnel, ~1.28 GB/call at ~55 MB/s):
  - Node-feature tables are internal DRAM tiles in bf16 (256 B rows =
    dma_gather's granule), gathered directly from the AllGather output.
    v2 shipped them as 10 ExternalOutputs (512 MB of donated zeros up +
    512 MB down per call) plus on-device bounce copies - all gone.
  - The edge selection matrices S (one [128 slot, 128 dst] fp8 block per
    128-edge chunk, scatter-add via TensorE matmul PSUM accumulation)
    are built ON DEVICE with one DVE tensor_scalar per chunk
    (iota==dst)*norm from compact [128, CTOT] dst/norm arrays
    (~0.9 MB/core up instead of 14.7 MB/core of prebuilt fp8 S).
  - T0 = x @ W1[3] is computed on device (49 extra matmuls + 1 more
    AllGather round: 6 total) instead of uploading 102 MB of host tables.
  - Gather indices ship compact [16, L/16] and are replicated to the
    [128, L/16] wrapped layout on device.
  - The jitted SPMD executable is cached across kernel() calls (a fresh
    jax.jit closure per call forced retrace + NEFF recompile each time),
    and NEFF compilation is disk-cached by BIR content hash.

v3.5+ steady-state refinements (the axon tunnel has ~85 ms RTT; every
blocking op pays it, so a call's floor is one blocking result fetch):
  - All inputs live device-resident as sharded jax.Arrays: graph-derived
    ones uploaded at runner construction, x/W-derived ones re-uploaded
    only when a crc32 content hash of the dynamic inputs changes.
  - The exec is dispatched optimistically with the resident dynamics so
    the input hash overlaps device execution; output fetch reads only
    core 0's shard (one round trip). No donation: the kernel fully
    writes its [1,1] output, so zero-seed buffers stay resident too.
  - dma_gather calls round-robin over 4 SWDGE queues, run 4-deep
    ring buffers per stream, and are emitted lo/hi-interleaved to match
    per-block consumption order (Tile keeps gpsimd program order, so
    emission order gates when each stream's gathers can start). Device
    exec ~2.7 ms vs 6.6 ms single-queue/2-deep/sequential, measured by
    pipelined async dispatch; wall-clock per call ~90 ms vs 20.8 s
    baseline is tunnel-RTT-bound.
  - U0 = h1 @ W2[3] is fused into the h1 hop loop per block, so its
    AllGather launches the moment the last h1 block lands.
  - S chunks are built 16-at-a-time with stride-0 broadcast-AP
    tensor_tensor ops; the per-layer bias rides into PSUM as a rank-1
    ones x bias matmul inside each block's accumulation group, so PReLU
    is 3 DVE ops reading PSUM directly. GIDX=1024 is the stable maximum:
    1536+ indices per gather call kills the remote worker.
"""
import hashlib
import math
import os
import shutil

import numpy as np
import ml_dtypes

BF = ml_dtypes.bfloat16

NCORES = 8
NNODES = 50000
FEAT = 128
KHOPS = 3
NBLK = 49                 # 128-row destination blocks per core
RPC = NBLK * 128          # rows per core (6272)
NPAD = NCORES * RPC       # padded node count (50176)
SPLIT = NPAD // 2         # 25088: table A/B split (int16 gather indices)
GIDX = 1024               # indices per dma_gather call
NAGT = 6                  # AllGather'd tables (T0,T1,T2,U0,U1,U2)

_CACHE: dict = {}

# Debug bisection: emit only the first K_STEPS stages (1..7); 7 = full program.
K_STEPS = int(os.environ.get("K_STEPS", "7"))


# ----------------------------------------------------------------------------
# Host preprocessing: graph -> per-core gather streams + chunk dst/norm arrays
# ----------------------------------------------------------------------------

def _preprocess(edge_index: np.ndarray):
    """Returns (struct, statics) where statics are the global (concatenated
    over cores along axis 0) graph-derived device inputs."""
    row = edge_index[0].astype(np.int64)
    col = edge_index[1].astype(np.int64)
    E = row.shape[0]

    deg = np.bincount(row, minlength=NNODES).astype(np.float64)
    dinv = np.where(deg > 0, 1.0 / np.sqrt(np.maximum(deg, 1.0)), 0.0)
    norm = (dinv[row] * dinv[col]).astype(np.float32)

    gb = row >> 7                       # global 128-row dst block (0..391)
    half = (col >= SPLIT).astype(np.int64)
    order = np.lexsort((half, gb))
    gb_s, half_s, col_s, row_s, norm_s = (
        gb[order], half[order], col[order], row[order], norm[order])

    # counts per (global block, half); chunk capacity = max over cores
    key = gb_s * 2 + half_s
    cnt = np.bincount(key, minlength=NCORES * NBLK * 2)
    cnt_lo = cnt[0::2].reshape(NCORES, NBLK)
    cnt_hi = cnt[1::2].reshape(NCORES, NBLK)
    CLO = np.maximum(1, np.ceil(cnt_lo.max(0) / 128).astype(np.int64))
    CHI = np.maximum(1, np.ceil(cnt_hi.max(0) / 128).astype(np.int64))

    LLO = int(CLO.sum()) * 128          # lo gather stream length (slots)
    LHI = int(CHI.sum()) * 128
    CTOT = int(CLO.sum() + CHI.sum())   # S chunks per hop

    starts = np.zeros(NCORES * NBLK * 2 + 1, np.int64)
    np.cumsum(cnt, out=starts[1:])

    # chunk bases shared by all cores
    sbase_lo = np.concatenate([[0], np.cumsum(CLO + CHI)])[:-1]
    sbase_hi = sbase_lo + CLO
    lobase = np.concatenate([[0], np.cumsum(CLO)])[:-1]  # lo stream chunks
    hibase = np.concatenate([[0], np.cumsum(CHI)])[:-1]

    # vectorized per-edge placement
    core_s = gb_s // NBLK
    b_s = gb_s % NBLK
    pos = np.arange(E) - starts[key]            # position within segment
    is_lo = half_s == 0
    part = pos % 128
    chunk = np.where(is_lo, sbase_lo[b_s], sbase_hi[b_s]) + pos // 128
    slot = np.where(is_lo, lobase[b_s], hibase[b_s]) * 128 + pos

    idx_lo_all = np.zeros((NCORES, LLO), np.int16)
    idx_hi_all = np.zeros((NCORES, LHI), np.int16)
    lo, hi = is_lo, ~is_lo
    idx_lo_all[core_s[lo], slot[lo]] = col_s[lo].astype(np.int16)
    idx_hi_all[core_s[hi], slot[hi]] = (col_s[hi] - SPLIT).astype(np.int16)
    dstv = np.full((NCORES, 128, CTOT), -1.0, np.float32)
    normv = np.zeros((NCORES, 128, CTOT), np.float32)
    dstv[core_s, part, chunk] = row_s & 127
    normv[core_s, part, chunk] = norm_s

    rows = np.arange(NPAD).reshape(NCORES, NBLK, 128)
    mk = (rows < NNODES).transpose(0, 2, 1).astype(np.float32).astype(BF)
    statics = dict(
        idx_lo=np.ascontiguousarray(
            idx_lo_all.reshape(NCORES, LLO // 16, 16).transpose(0, 2, 1)
        ).reshape(NCORES * 16, LLO // 16),
        idx_hi=np.ascontiguousarray(
            idx_hi_all.reshape(NCORES, LHI // 16, 16).transpose(0, 2, 1)
        ).reshape(NCORES * 16, LHI // 16),
        dstv=dstv.reshape(NCORES * 128, CTOT),
        normv=normv.reshape(NCORES * 128, CTOT),
        maskc=np.ascontiguousarray(mk).reshape(NCORES * 128, NBLK),
    )
    struct = dict(CLO=tuple(int(v) for v in CLO), CHI=tuple(int(v) for v in CHI),
                  LLO=LLO, LHI=LHI, CTOT=CTOT)
    return struct, statics


# ----------------------------------------------------------------------------
# Bass program
# ----------------------------------------------------------------------------

def _build_program(struct):
    import concourse.bacc as bacc
    import concourse.mybir as mybir
    import concourse.tile as tile

    CLO, CHI = struct["CLO"], struct["CHI"]
    LLO, LHI, CTOT = struct["LLO"], struct["LHI"], struct["CTOT"]
    f32 = mybir.dt.float32
    bf16 = mybir.dt.bfloat16
    fp8 = mybir.dt.float8e4
    i16 = mybir.dt.int16

    nc = bacc.Bacc("TRN2", target_bir_lowering=False, debug=False,
                   num_devices=NCORES, dynamic_dma_scratch_size=32768,
                   num_swdge_queues=4)

    P = {}
    def param(name, shape, dt):
        P[name] = nc.declare_dram_parameter(name, list(shape), dt, isOutput=False)
        return P[name]

    param("idx_lo", [16, LLO // 16], i16)
    param("idx_hi", [16, LHI // 16], i16)
    param("dstv", [128, CTOT], f32)
    param("normv", [128, CTOT], f32)
    param("xT", [128, RPC], bf16)
    param("w1", [128, KHOPS + 1, 128], bf16)
    param("w2", [128, KHOPS + 1, 128], bf16)
    param("b1r", [1, 128], f32)
    param("b2r", [1, 128], f32)
    param("a1c", [128, 1], f32)
    param("a2c", [128, 1], f32)
    param("wout", [128, 1], f32)
    param("boutc", [1, 1], f32)
    param("maskc", [128, NBLK], bf16)
    out_ext = nc.declare_dram_parameter("out", [1, 1], f32, isOutput=True)

    RG = [list(range(NCORES))]
    NLO_CALLS = math.ceil(LLO / GIDX)
    NHI_CALLS = math.ceil(LHI / GIDX)

    with tile.TileContext(nc) as tc:
        with (
            tc.tile_pool(name="const", bufs=1) as cpool,
            tc.tile_pool(name="glo", bufs=4) as glop,
            tc.tile_pool(name="ghi", bufs=4) as ghip,
            tc.tile_pool(name="work", bufs=4) as wkp,
            tc.tile_pool(name="eqp", bufs=1) as eqp,
            tc.tile_pool(name="ps", bufs=4, space="PSUM") as psp,
            tc.tile_pool(name="pst", bufs=2, space="PSUM") as pst,
            tc.tile_pool(name="psx", bufs=1, space="PSUM") as psx,
            tc.tile_pool(name="dram", bufs=1, space="DRAM") as drp,
        ):
            # ---- resident constants ----
            def cload(name, shape, dt, tag):
                t = cpool.tile(shape, dt, tag=tag)
                nc.sync.dma_start(out=t[:], in_=P[name][:])
                return t

            dstv_t = cload("dstv", [128, CTOT, 1], f32, "dstv")
            normv_t = cload("normv", [128, CTOT, 1], f32, "normv")
            xT_t = cload("xT", [128, RPC], bf16, "xT")
            w1_t = cload("w1", [128, KHOPS + 1, 128], bf16, "w1")
            w2_t = cload("w2", [128, KHOPS + 1, 128], bf16, "w2")
            b1r_t = cload("b1r", [1, 128], f32, "b1r")
            b2r_t = cload("b2r", [1, 128], f32, "b2r")
            a1c_t = cload("a1c", [128, 1], f32, "a1c")
            a2c_t = cload("a2c", [128, 1], f32, "a2c")
            wout_t = cload("wout", [128, 1], f32, "wout")
            bout_t = cload("boutc", [1, 1], f32, "bout")
            maskc_t = cload("maskc", [128, NBLK], bf16, "maskc")
            h1T_t = cpool.tile([128, RPC], bf16, tag="h1T")

            # gather index tables: replicate [16, L/16] -> wrapped [128, L/16]
            idxlo_t = cpool.tile([128, LLO // 16], i16, tag="idxlo")
            idxhi_t = cpool.tile([128, LHI // 16], i16, tag="idxhi")
            for g in range(8):
                nc.sync.dma_start(out=idxlo_t[16 * g:16 * (g + 1), :],
                                  in_=P["idx_lo"][:])
                nc.sync.dma_start(out=idxhi_t[16 * g:16 * (g + 1), :],
                                  in_=P["idx_hi"][:])

            # ---- on-device constant builds ----
            from concourse.bass import broadcast_tensor_aps
            iota_t = cpool.tile([128, 1, 128], f32, tag="iota")
            nc.gpsimd.iota(out=iota_t[:], pattern=[[0, 1], [1, 128]], base=0,
                           channel_multiplier=0,
                           allow_small_or_imprecise_dtypes=True)
            lane_t = cpool.tile([128, 1], f32, tag="lane")
            nc.gpsimd.iota(out=lane_t[:], pattern=[[0, 1]], base=0,
                           channel_multiplier=1,
                           allow_small_or_imprecise_dtypes=True)
            ident_t = cpool.tile([128, 128], f32, tag="ident")
            nc.vector.tensor_scalar(out=ident_t[:], in0=iota_t[:, 0, :],
                                    scalar1=lane_t[:, 0:1], scalar2=None,
                                    op0=mybir.AluOpType.is_equal)

            # S chunks: S[p, si, j] = (j == dstv[p, si]) * normv[p, si],
            # built SGRP chunks per pass via stride-0 broadcast APs.
            SGRP = 16
            smat_t = cpool.tile([128, CTOT, 128], fp8, tag="smat")
            for g in range(0, CTOT, SGRP):
                ng = min(SGRP, CTOT - g)
                eq_t = eqp.tile([128, SGRP, 128], bf16, tag="eq")
                a, b = broadcast_tensor_aps(iota_t[:, 0:1, :],
                                            dstv_t[:, g:g + ng, :])
                nc.vector.tensor_tensor(out=eq_t[:, :ng, :], in0=a, in1=b,
                                        op=mybir.AluOpType.is_equal)
                c, d = broadcast_tensor_aps(eq_t[:, :ng, :],
                                            normv_t[:, g:g + ng, :])
                nc.vector.tensor_tensor(out=smat_t[:, g:g + ng, :],
                                        in0=c, in1=d,
                                        op=mybir.AluOpType.mult)

            # ones row for folding the bias add into the PSUM matmul group:
            # matmul(lhsT=ones[1,128], rhs=b[1,128]) adds b[j] to every row.
            ones_t = cpool.tile([1, 128], f32, tag="ones")
            nc.vector.memset(ones_t[:], 1.0)

            # ---- DRAM internals ----
            agbufs = [drp.tile([NPAD, FEAT], bf16, tag=f"agbuf{i}",
                               name=f"agbuf{i}", addr_space="Shared")
                      for i in range(NAGT)]
            bounces = [drp.tile([RPC, FEAT], bf16, tag=f"bounce{i}",
                                name=f"bounce{i}") for i in range(NAGT)]
            red_in = drp.tile([1, 1], f32, tag="red_in")
            red_out = drp.tile([1, 1], f32, tag="red_out", addr_space="Shared")

            def emit_gathers(srcA, srcB):
                # Interleave lo/hi emission to match per-block consumption
                # order (each block consumes its lo chunks, then its hi ones).
                lo_bufs, hi_bufs = [], []
                q = 0
                for call in range(max(NLO_CALLS, NHI_CALLS)):
                    if call < NLO_CALLS:
                        n = min(GIDX, LLO - call * GIDX)
                        gt = glop.tile([128, GIDX // 128, FEAT], bf16,
                                       tag="glo")
                        nc.gpsimd.dma_gather(
                            out_ap=gt[:, : n // 128, :],
                            in_ap=srcA,
                            idxs_ap=idxlo_t[:, call * (GIDX // 16):
                                            call * (GIDX // 16) + n // 16],
                            num_idxs=n, num_idxs_reg=n, elem_size=FEAT,
                            queue_num=q % 4)
                        q += 1
                        lo_bufs.append(gt)
                    if call < NHI_CALLS:
                        n = min(GIDX, LHI - call * GIDX)
                        gt = ghip.tile([128, GIDX // 128, FEAT], bf16,
                                       tag="ghi")
                        nc.gpsimd.dma_gather(
                            out_ap=gt[:, : n // 128, :],
                            in_ap=srcB,
                            idxs_ap=idxhi_t[:, call * (GIDX // 16):
                                            call * (GIDX // 16) + n // 16],
                            num_idxs=n, num_idxs_reg=n, elem_size=FEAT,
                            queue_num=q % 4)
                        q += 1
                        hi_bufs.append(gt)
                return lo_bufs, hi_bufs

            def hop_blocks(srcA, srcB, w_tile, k, lhsT_tile, bias_row=None):
                """Yield (b, psum_tile): x@W[k] [+ b] + A·t accumulated per
                block. bias_row [1,128] is folded in via a rank-1 matmul."""
                lo_bufs, hi_bufs = emit_gathers(srcA, srcB)
                lo_sub = hi_sub = s_pos = 0
                for b in range(NBLK):
                    pt = psp.tile([128, FEAT], f32, tag="blk", space="PSUM")
                    first = True
                    for j in range(CLO[b]):
                        gs, si = lo_sub + j, s_pos + j
                        nc.tensor.matmul(
                            out=pt[:],
                            lhsT=smat_t[:, si, :],
                            rhs=lo_bufs[gs // (GIDX // 128)][:, gs % (GIDX // 128), :],
                            start=first, stop=False)
                        first = False
                    s_pos += CLO[b]
                    for j in range(CHI[b]):
                        gs, si = hi_sub + j, s_pos + j
                        nc.tensor.matmul(
                            out=pt[:],
                            lhsT=smat_t[:, si, :],
                            rhs=hi_bufs[gs // (GIDX // 128)][:, gs % (GIDX // 128), :],
                            start=first, stop=False)
                        first = False
                    s_pos += CHI[b]
                    lo_sub += CLO[b]
                    hi_sub += CHI[b]
                    if bias_row is not None:
                        nc.tensor.matmul(
                            out=pt[:], lhsT=ones_t[0:1, :],
                            rhs=bias_row[0:1, :], start=first, stop=False)
                        first = False
                    nc.tensor.matmul(
                        out=pt[:], lhsT=lhsT_tile[:, 128 * b:128 * (b + 1)],
                        rhs=w_tile[:, k, :], start=first, stop=True)
                    yield b, pt

            def dense_blocks(w_tile, k, lhsT_tile):
                for b in range(NBLK):
                    pt = psp.tile([128, FEAT], f32, tag="blk", space="PSUM")
                    nc.tensor.matmul(
                        out=pt[:], lhsT=lhsT_tile[:, 128 * b:128 * (b + 1)],
                        rhs=w_tile[:, k, :], start=True, stop=True)
                    yield b, pt

            def store_and_gather_table(block_iter, ti):
                for b, pt in block_iter:
                    cb = wkp.tile([128, FEAT], bf16, tag="cb")
                    nc.any.tensor_copy(out=cb[:], in_=pt[:])
                    nc.sync.dma_start(out=bounces[ti][128 * b:128 * (b + 1), :],
                                      in_=cb[:])
                nc.gpsimd.collective_compute(
                    "AllGather", mybir.AluOpType.bypass, replica_groups=RG,
                    ins=[bounces[ti].opt()], outs=[agbufs[ti].opt()])
                return agbufs[ti][0:SPLIT, :], agbufs[ti][SPLIT:NPAD, :]

            def prelu(pt, ac_t, dst_ap):
                """dst = max(pt,0) + a*min(pt,0); bias already in PSUM."""
                pos = wkp.tile([128, 128], f32, tag="tmp1")
                neg = wkp.tile([128, 128], f32, tag="tmp2")
                nc.vector.tensor_scalar(out=neg[:], in0=pt[:], scalar1=0.0,
                                        scalar2=ac_t[:, 0:1],
                                        op0=mybir.AluOpType.min,
                                        op1=mybir.AluOpType.mult)
                nc.vector.tensor_scalar(out=pos[:], in0=pt[:],
                                        scalar1=0.0, scalar2=None,
                                        op0=mybir.AluOpType.max)
                nc.vector.tensor_tensor(out=dst_ap, in0=pos[:], in1=neg[:],
                                        op=mybir.AluOpType.add)

            # ---- Layer 1, Horner: T0 = x @ W1[3] on device ----
            if K_STEPS >= 1:
                tA, tB = store_and_gather_table(dense_blocks(w1_t, 3, xT_t), 0)
            # hop 1: t = x @ W1[2] + A T0 -> table 1
            if K_STEPS >= 2:
                tA, tB = store_and_gather_table(
                    hop_blocks(tA, tB, w1_t, 2, xT_t), 1)
            # hop 2: t = x @ W1[1] + A t -> table 2
            if K_STEPS >= 3:
                tA, tB = store_and_gather_table(
                    hop_blocks(tA, tB, w1_t, 1, xT_t), 2)
            # hop 3: h1 = prelu(x @ W1[0] + A t + b1); keep h1T on chip.
            # Layer 2's U0 = h1 @ W2[3] is fused per block so its AllGather
            # can start the moment the last h1 block lands.
            if K_STEPS >= 4:
                for b, pt in hop_blocks(tA, tB, w1_t, 0, xT_t, bias_row=b1r_t):
                    h1f = wkp.tile([128, 128], f32, tag="h1f")
                    prelu(pt, a1c_t, h1f[:])
                    ptr = pst.tile([128, 128], f32, tag="tr", space="PSUM")
                    nc.tensor.transpose(out=ptr[:], in_=h1f[:],
                                        identity=ident_t[:])
                    nc.any.tensor_copy(out=h1T_t[:, 128 * b:128 * (b + 1)],
                                       in_=ptr[:])
                    if K_STEPS >= 5:
                        pt2 = psp.tile([128, FEAT], f32, tag="blk",
                                       space="PSUM")
                        nc.tensor.matmul(
                            out=pt2[:], lhsT=h1T_t[:, 128 * b:128 * (b + 1)],
                            rhs=w2_t[:, 3, :], start=True, stop=True)
                        cb = wkp.tile([128, FEAT], bf16, tag="cb")
                        nc.any.tensor_copy(out=cb[:], in_=pt2[:])
                        nc.sync.dma_start(
                            out=bounces[3][128 * b:128 * (b + 1), :],
                            in_=cb[:])

            # ---- Layer 2, Horner ----
            # u = h1 @ W2[3] -> table 3 (blocks stored above; gather it)
            if K_STEPS >= 5:
                nc.gpsimd.collective_compute(
                    "AllGather", mybir.AluOpType.bypass, replica_groups=RG,
                    ins=[bounces[3].opt()], outs=[agbufs[3].opt()])
                tA, tB = agbufs[3][0:SPLIT, :], agbufs[3][SPLIT:NPAD, :]
            # u = h1 @ W2[2] + A u -> table 4
            if K_STEPS >= 6:
                tA, tB = store_and_gather_table(
                    hop_blocks(tA, tB, w2_t, 2, h1T_t), 4)
            if K_STEPS >= 7:
                # u = h1 @ W2[1] + A u -> table 5
                tA, tB = store_and_gather_table(
                    hop_blocks(tA, tB, w2_t, 1, h1T_t), 5)
                # h2 = prelu(h1 @ W2[0] + A u + b2); pool
                pool_ps = psx.tile([128, 1], f32, tag="pool", space="PSUM")
                for b, pt in hop_blocks(tA, tB, w2_t, 0, h1T_t,
                                        bias_row=b2r_t):
                    h2b = wkp.tile([128, 128], bf16, tag="h2b")
                    prelu(pt, a2c_t, h2b[:])
                    nc.tensor.matmul(out=pool_ps[:], lhsT=h2b[:],
                                     rhs=maskc_t[:, b:b + 1],
                                     start=(b == 0), stop=(b == NBLK - 1))

                # ---- finale: (pool @ Wout), AllReduce, + bout ----
                pv = wkp.tile([128, 1], f32, tag="pv")
                nc.any.tensor_copy(out=pv[:], in_=pool_ps[:])
                fin_ps = psx.tile([1, 1], f32, tag="fin", space="PSUM")
                nc.tensor.matmul(out=fin_ps[:], lhsT=pv[:], rhs=wout_t[:],
                                 start=True, stop=True)
                sfin = wkp.tile([1, 1], f32, tag="sfin")
                nc.any.tensor_copy(out=sfin[:], in_=fin_ps[:])
                nc.sync.dma_start(out=red_in[:], in_=sfin[:])
                nc.gpsimd.collective_compute(
                    "AllReduce", mybir.AluOpType.add, replica_groups=RG,
                    ins=[red_in.opt()], outs=[red_out.opt()])
                sred = wkp.tile([1, 1], f32, tag="sred")
                nc.sync.dma_start(out=sred[:], in_=red_out[:])
                sout = wkp.tile([1, 1], f32, tag="sout")
                nc.vector.tensor_tensor(out=sout[:], in0=sred[:],
                                        in1=bout_t[:],
                                        op=mybir.AluOpType.add)
                nc.sync.dma_start(out=out_ext[:], in_=sout[:])
            else:
                souT = wkp.tile([1, 1], f32, tag="souT")
                nc.vector.tensor_copy(out=souT[:], in_=bout_t[:])
                nc.sync.dma_start(out=out_ext[:], in_=souT[:])

    nc.finalize()
    return nc


# ----------------------------------------------------------------------------
# Cached SPMD runner (mirrors bass2jax.run_bass_via_pjrt, but the jitted
# executable is built once and reused across calls)
# ----------------------------------------------------------------------------

def _install_neff_disk_cache():
    import concourse.bass2jax as b2j
    if getattr(b2j.compile_bir_kernel, "_neff_cached", False):
        return
    orig = b2j.compile_bir_kernel

    def cached(bir_json, tmpdir, neff_name="file.neff"):
        cpath = None
        try:
            h = hashlib.sha256(bir_json).hexdigest()
            cdir = "/tmp/bass_neff_cache"
            os.makedirs(cdir, exist_ok=True)
            cpath = os.path.join(cdir, h + ".neff")
            if os.path.exists(cpath):
                dst = os.path.join(tmpdir, neff_name)
                shutil.copyfile(cpath, dst)
                return dst
        except OSError:
            cpath = None
        p = orig(bir_json, tmpdir, neff_name)
        if cpath is not None:
            try:
                shutil.copyfile(p, cpath + f".tmp{os.getpid()}")
                os.replace(cpath + f".tmp{os.getpid()}", cpath)
            except OSError:
                pass
        return p

    cached._neff_cached = True
    b2j.compile_bir_kernel = cached


class _Runner:
    """run_bass_via_pjrt with the jit closure hoisted out of the call.

    ``static`` inputs (pure functions of the graph structure) are uploaded
    once at construction as sharded jax.Arrays; per-call uploads are only
    the x/W-derived tensors.
    """

    def __init__(self, nc, n_cores, static_maps=None):
        import jax
        import time
        import concourse.bass2jax as b2j
        import concourse.mybir as mybir
        from jax.sharding import Mesh, PartitionSpec, NamedSharding
        try:
            from jax.experimental.shard_map import shard_map
        except ImportError:  # newer jax
            from jax.sharding import shard_map

        _install_neff_disk_cache()
        b2j.install_neuronx_cc_hook()
        try:  # fresh-process first calls skip XLA compile on cache hits
            jax.config.update("jax_compilation_cache_dir", "/tmp/jax_ccache")
            jax.config.update("jax_persistent_cache_min_compile_time_secs", 0.0)
            jax.config.update("jax_persistent_cache_min_entry_size_bytes", 0)
        except Exception:
            pass
        assert nc.dbg_addr is None or not nc.dbg_callbacks
        self.n_cores = n_cores
        self.stats = {}
        self.dbg_name = None
        if nc.dbg_addr is not None:
            self.dbg_name = nc.dbg_addr.name

        partition_name = (nc.partition_id_tensor.name
                          if nc.partition_id_tensor else None)
        in_names, out_names, out_avals, zero_shapes = [], [], [], []
        for alloc in nc.m.functions[0].allocations:
            if not isinstance(alloc, mybir.MemoryLocationSet):
                continue
            name = alloc.memorylocations[0].name
            if alloc.kind == "ExternalInput":
                if name != partition_name:
                    in_names.append(name)
            elif alloc.kind == "ExternalOutput":
                shape = tuple(alloc.tensor_shape)
                dtype = mybir.dt.np(alloc.dtype)
                out_names.append(name)
                out_avals.append(jax.core.ShapedArray(shape, dtype))
                zero_shapes.append((shape, dtype))
        self.in_names = in_names
        self.out_names = out_names
        self.out_avals = out_avals
        self.zero_shapes = zero_shapes
        n_params = self.n_params = len(in_names)
        n_outs = len(out_names)
        all_in_names = list(in_names) + list(out_names)
        if partition_name is not None:
            all_in_names.append(partition_name)
        donate = tuple(range(n_params, n_params + n_outs))

        def _body(*args):
            operands = list(args)
            if partition_name is not None:
                operands.append(b2j.partition_id_tensor())
            outs = b2j._bass_exec_p.bind(
                *operands,
                out_avals=tuple(out_avals),
                in_names=tuple(all_in_names),
                out_names=tuple(out_names),
                lowering_input_output_aliases=(),
                sim_require_finite=True,
                sim_require_nnan=True,
                nc=nc,
            )
            return tuple(outs)

        devices = jax.devices()[:n_cores]
        assert len(devices) == n_cores
        mesh = Mesh(np.asarray(devices), ("core",))
        in_specs = (PartitionSpec("core"),) * (n_params + n_outs)
        out_specs = (PartitionSpec("core"),) * n_outs
        # No donation: the kernel fully writes its single [1,1] output, so
        # the zero "output seed" buffers need not alias — they can stay
        # device-resident instead of being re-uploaded and consumed per call.
        del donate
        self.sharded = jax.jit(
            shard_map(_body, mesh=mesh, in_specs=in_specs,
                      out_specs=out_specs, check_rep=False),
            keep_unused=True)

        # Pre-upload graph-structure inputs once, sharded to match in_specs.
        self._time = time.perf_counter
        self._jax = jax
        self._sh = NamedSharding(mesh, PartitionSpec("core"))
        self.resident = {}
        if static_maps:
            self.resident = self.upload(static_maps)
        self.zeros_res = list(self.upload({
            f"__z{i}": np.zeros((n_cores * s[0], *s[1:]), d)
            for i, (s, d) in enumerate(self.zero_shapes)
        }).values())
        if self.dbg_name is not None:
            self.resident.update(self.upload(
                {self.dbg_name: np.zeros((n_cores, 2), np.uint32)}))

    def upload(self, arrays):
        """device_put a {name: global array} dict sharded along axis 0."""
        out = {name: self._jax.device_put(np.asarray(a), self._sh)
               for name, a in arrays.items()}
        self._jax.block_until_ready(list(out.values()))
        return out

    def __call__(self, dyn):
        """dyn: {name: global [n_cores*d0, ...] array} for non-resident inputs."""
        n = self.n_cores
        t0 = self._time()
        concat_in = [self.resident[nm] if nm in self.resident else dyn[nm]
                     for nm in self.in_names]
        t1 = self._time()
        out_arrs = self.sharded(*concat_in, *self.zeros_res)
        t2 = self._time()
        res = [
            {name: np.asarray(out_arrs[i]).reshape(n, *self.out_avals[i].shape)[c]
             for i, name in enumerate(self.out_names)}
            for c in range(n)
        ]
        t3 = self._time()
        self.stats = {"concat_s": t1 - t0, "dispatch_s": t2 - t1,
                      "fetch_s": t3 - t2}
        return res

    def start(self, dyn):
        """Async dispatch; returns the output futures without blocking."""
        concat_in = [self.resident[nm] if nm in self.resident else dyn[nm]
                     for nm in self.in_names]
        return self.sharded(*concat_in, *self.zeros_res)

    def finish_first_shard(self, out_arrs, name):
        """Fetch only core 0's shard of output `name` (1 round trip)."""
        t1 = self._time()
        i = self.out_names.index(name)
        val = np.asarray(out_arrs[i].addressable_shards[0].data)
        self.stats = {"concat_s": 0.0, "dispatch_s": 0.0,
                      "fetch_s": self._time() - t1}
        return val

    def call_first_shard(self, dyn, name):
        return self.finish_first_shard(self.start(dyn), name)


# ----------------------------------------------------------------------------
# Global (pre-concatenated) input arrays
# ----------------------------------------------------------------------------

def _dynamic_inputs(inputs):
    x = np.asarray(inputs["x"], np.float32)
    W1 = np.asarray(inputs["W1"], np.float32)
    W2 = np.asarray(inputs["W2"], np.float32)
    b1 = np.asarray(inputs["b1"], np.float32)
    b2 = np.asarray(inputs["b2"], np.float32)
    a1 = np.asarray(inputs["a1"], np.float32)
    a2 = np.asarray(inputs["a2"], np.float32)
    Wout = np.asarray(inputs["Wout"], np.float32)
    bout = np.asarray(inputs["bout"], np.float32)
    K1 = KHOPS + 1

    xpad = np.zeros((NPAD, FEAT), np.float32)
    xpad[:NNODES] = x
    xT = xpad.T.astype(BF)                       # [128, NPAD], C-contiguous
    xTg = np.ascontiguousarray(
        xT.reshape(128, NCORES, RPC).transpose(1, 0, 2)
    ).reshape(NCORES * 128, RPC)

    w1s = np.ascontiguousarray(W1.transpose(1, 0, 2)).astype(BF)
    w2s = np.ascontiguousarray(W2.transpose(1, 0, 2)).astype(BF)
    rep = lambda a: np.ascontiguousarray(                     # noqa: E731
        np.broadcast_to(a, (NCORES, *a.shape))).reshape(
            NCORES * a.shape[0], *a.shape[1:])
    return dict(
        xT=xTg,
        w1=rep(w1s), w2=rep(w2s),
        b1r=rep(b1.reshape(1, 128).astype(np.float32)),
        b2r=rep(b2.reshape(1, 128).astype(np.float32)),
        a1c=np.full((NCORES * 128, 1), float(a1[0]), np.float32),
        a2c=np.full((NCORES * 128, 1), float(a2[0]), np.float32),
        wout=rep(Wout.reshape(128, 1).astype(np.float32)),
        boutc=np.full((NCORES, 1), float(bout[0]), np.float32),
    )


# ----------------------------------------------------------------------------
# Entry point
# ----------------------------------------------------------------------------

_DYN_NAMES = ("x", "W1", "b1", "a1", "W2", "b2", "a2", "Wout", "bout")


def _hash_dynamic(inputs):
    import zlib
    crc = 0
    parts = []
    for name in _DYN_NAMES:
        a = np.ascontiguousarray(np.asarray(inputs[name]))
        crc = zlib.crc32(a.view(np.uint8).reshape(-1), crc)
        parts.append(f"{name}{a.shape}{a.dtype}")
    return f"{crc:08x}|" + "|".join(parts)


def kernel(**inputs) -> np.ndarray:
    edge_index = np.asarray(inputs["edge_index"])
    ekey = hashlib.sha1(edge_index.tobytes()).hexdigest()
    entry = _CACHE.get(ekey)
    if entry is None:
        struct, statics = _preprocess(edge_index)
        nc = _build_program(struct)
        entry = {"struct": struct, "nc": nc, "runner": "uninit",
                 "statics": statics, "dkey": None}
        _CACHE[ekey] = entry

    if entry["runner"] == "uninit":
        try:
            entry["runner"] = _Runner(entry["nc"], NCORES, entry["statics"])
        except Exception:
            import traceback
            traceback.print_exc()
            entry["runner"] = None

    runner = entry["runner"]
    if runner is not None:
        try:
            # Optimistic dispatch with the resident dynamics overlaps the
            # input hash with device execution; on mismatch the speculative
            # run is discarded and the fresh inputs are uploaded.
            fut = None
            if entry["dkey"] is not None:
                fut = runner.start(entry["dyn_res"])
            dkey = _hash_dynamic(inputs)
            if entry["dkey"] != dkey:
                fut = None
                entry["dyn_res"] = runner.upload(_dynamic_inputs(inputs))
                entry["dkey"] = dkey
                # Throwaway exec: absorbs first-exec-after-upload races
                # (one transient off-by-5e-4 first-call result was observed).
                runner.call_first_shard(entry["dyn_res"], "out")
            if fut is None:
                fut = runner.start(entry["dyn_res"])
            out = runner.finish_first_shard(fut, "out")
            return np.asarray(out, np.float32).reshape(1, 1)
        except Exception:
            import traceback
            traceback.print_exc()
            entry["runner"] = None  # don't retry the fast path
            entry["dkey"] = None

    from concourse.bass_utils import run_bass_kernel_spmd
    full = {**entry["statics"], **_dynamic_inputs(inputs)}
    maps = [
        {k: v[c * (v.shape[0] // NCORES):(c + 1) * (v.shape[0] // NCORES)]
         for k, v in full.items()}
        for c in range(NCORES)
    ]
    res = run_bass_kernel_spmd(nc=entry["nc"], in_maps=maps,
                               core_ids=list(range(NCORES)))
    return np.asarray(res.results[0]["out"], np.float32)
